# revision 51
# baseline (speedup 1.0000x reference)
"""Degraded bicycle rollout kernel for Trainium2 (8 NeuronCores, data-parallel on batch).

Structure (v4):
  - Speed recurrence s' = max(s + a, 0) (eps-free, x10-scaled) solved by ONE
    hardware tensor_tensor_scan per chunk over an 81-slot padded layout:
    slot 0 of each rollout holds -60000 (clamps the running state to 0 =
    segment reset), slot 1 is pre-biased with 10*s0. psi0 is folded into
    the yaw cumsum the same way (first-slot bias on a scratch copy).
  - Controls DMA'd as fp16 channel-major per chunk, so wave 1 is two ACT
    ops (tanh(u0), tanh(0.5*[u1 u2])); sigmoid comes from the half-angle
    identity folded into the following TSP scalars.
  - Wave 1 is Tanh-only and shares the silu_and_others ACT table with Sin;
    only the Arctan switches tables (2 LoadActFuncSet total). All wave-1
    ACT runs before wave 2 (the arctan dep enforces it), so the first
    chunk's trig chain starts as early as possible.
  - Wave 2 is emitted strictly chunk-sequentially (engines are in-order;
    clustering a chunk's chain finishes chunk 0 - and its output DMA -
    first). All navigation state lives in dense tiles; staging writes are
    terminal copies, so a chunk blocked on a staging buffer still runs its
    full compute chain during earlier output DMAs.
  - fp16 intermediates give DVE 2x/4x perf modes where supported.

Layout per core: 4096 rollouts = 128 partitions x 32; partition p holds
rollouts p*32..p*32+31 of batch p//2. Free dim rollout-major f = n*80 + t.
"""

import sys

sys.path.insert(0, "/opt/trn_rl_repo")

import numpy as np

B, L, H = 512, 64, 80
NCORES = 8
BC = B // NCORES          # 64 batches per core
R = BC * L                # 4096 rollouts per core
P = 128
NPT = R // P              # 32 rollouts per partition
F = NPT * H               # 2560 elements per partition
HP1 = H + 1
CW = 12                   # output channels
SW = HP1 * CW             # 972 staging words per rollout
G = 4                     # chunks
NG = NPT // G             # 8 rollouts per partition per chunk
CF = NG * H               # 640 free elems per chunk
CFP = NG * (H + 1)        # 648 padded elems per chunk
CHW = NG * SW             # 7776 staging words per partition per chunk
DT = 0.1
WB = 2.8
PI = float(np.pi)
PAD = -60000.0            # fp16-safe segment-reset value
SINSC = 0.999999          # keeps wrapped args strictly inside [-pi, pi]

_BUILT = None


def _build_kernel():
    import concourse.bass as bass
    import concourse.bacc as bacc
    import concourse.mybir as mybir
    from concourse.tile import TileContext
    from concourse.tile_rust import add_dep_helper

    f32 = mybir.dt.float32
    f16 = mybir.dt.float16
    A = mybir.AluOpType
    AF = mybir.ActivationFunctionType

    # The act-table insertion pass is first-fit over act_info.json order.
    # Our op mix (tanh/sin/arctan/identity) is covered 2-load by
    # silu_and_others (via the dummy Silu below) + trig_and_small, but
    # first-fit would send the first Arctan to sigmoid_and_others (earlier
    # index, lacks sin) and force a third load. Hide Arctan from that set
    # during this build only - indices (= emitted act_func_set_ids) are
    # unchanged, so the compiled program stays fully valid.
    import concourse.hw_specs as _hws

    _orig_tables = _hws.get_activation_tables

    def _patched_tables(arch):
        d = dict(_orig_tables(arch))
        s = set(d.get("sigmoid_and_others", set()))
        s.discard(mybir.ActivationFunctionType.Arctan)
        d["sigmoid_and_others"] = s
        return d

    nc = bacc.Bacc(None, target_bir_lowering=False)
    ctrl_d = nc.declare_dram_parameter("ctrl", [P, NPT * H * 3], f16, isOutput=False)
    x0_d = nc.declare_dram_parameter("x0p", [P, 12], f32, isOutput=False)
    scal_d = nc.declare_dram_parameter("scal", [P, 16], f32, isOutput=False)
    out_d = nc.declare_dram_parameter("out", [P, NPT * SW], f32, isOutput=True)

    with TileContext(nc) as tc:
        v = nc.vector
        sc = nc.scalar
        gp = nc.gpsimd
        sy = nc.sync

        with tc.tile_pool(name="pers", bufs=1) as pp, \
             tc.tile_pool(name="ctrlp", bufs=4) as ctp, \
             tc.tile_pool(name="w1a", bufs=2) as w1p, \
             tc.tile_pool(name="lng", bufs=4) as lg, \
             tc.tile_pool(name="lg2", bufs=2) as lg2, \
             tc.tile_pool(name="sht", bufs=2) as sh, \
             tc.tile_pool(name="stgp", bufs=2) as sp:

            # ---------- persistent scalars / constants ----------
            x0s = pp.tile([P, 12], f32, tag="x0s")
            scal = pp.tile([P, 16], f32, tag="scal")
            steer = scal[:, 0:1]
            lo75 = scal[:, 3:4]      # -7.5*friction  (x10 accDT bound)
            f981 = scal[:, 4:5]      # 9.81*friction
            s0x10 = scal[:, 5:6]     # 10*sqrt(vx0^2+vy0^2+1e-6)
            px0 = scal[:, 7:8]
            py0 = scal[:, 8:9]
            vx010 = scal[:, 9:10]    # 10*vx0
            vy010 = scal[:, 10:11]   # 10*vy0
            psi010 = scal[:, 11:12]  # psi0/DT
            hbrake = scal[:, 12:13]  # 0.5*brake
            hthr = scal[:, 13:14]    # 0.5*thr
            hpiv = scal[:, 14:15]    # pi/2

            # ---------- input DMAs: ctrl chunk 0 first (critical path) ----
            CTW = NG * H * 3
            ctiles = []
            ct0 = ctp.tile([P, CTW], f16, tag="ctrl")
            sy.dma_start(out=ct0[:], in_=ctrl_d[:, 0:CTW])
            ctiles.append(ct0)
            sy.dma_start(out=x0s[:], in_=x0_d[:])
            sy.dma_start(out=scal[:], in_=scal_d[:])
            for g in range(1, G):
                ct = ctp.tile([P, CTW], f16, tag="ctrl")
                sy.dma_start(out=ct[:], in_=ctrl_d[:, g * CTW:(g + 1) * CTW])
                ctiles.append(ct)

            # Dummy Silu: the table pass is first-fit, so the first ACT op
            # determines the loaded set. Silu's first-containing set is
            # silu_and_others = {silu, tanh, sin, identity, copy, ...} which
            # covers everything here except Arctan -> the whole sin chain
            # runs without waiting for a table switch.
            dmy = pp.tile([P, 1], f32, tag="dmy")
            v.memset(dmy[:], 0.0)
            sc.activation(dmy[:], dmy[:], AF.Silu)

            maskc = pp.tile([P, CF], f16, tag="maskc")
            v.memset(maskc[:], 1.0)
            mc3 = maskc[:].rearrange("p (n h) -> p n h", n=NG)
            v.memset(mc3[:, :, 0:1], 0.0)

            zeros = pp.tile([P, CFP], f16, tag="zeros")
            v.memset(zeros[:], 0.0)

            apad = pp.tile([P, NPT * (H + 1)], f16, tag="apad")
            ap3 = apad[:].rearrange("p (n h) -> p n h", n=NPT)
            v.memset(ap3[:, :, 0:1], PAD)
            s2pad = pp.tile([P, NPT * (H + 1)], f16, tag="s2pad")
            s2p3 = s2pad[:].rearrange("p (n h) -> p n h", n=NPT)

            st = [dict() for _ in range(G)]
            first_bt = [None]

            def r3(t):
                return t[:].rearrange("p (n h) -> p n h", n=NG)

            # ---------- wave 1 ACT: all chunks (Tanh-only phase; u1/u2 are
            # host-prescaled by 0.5 so each chunk is ONE tanh op) ----------
            last_w1_act = None
            talls = []
            for g in range(G):
                tall = w1p.tile([P, 3 * CF], f16, tag="tall", bufs=4)
                i = sc.activation(tall[:], ctiles[g][:], AF.Tanh)
                last_w1_act = i
                talls.append(tall)

            # wave-1 DVE clusters are emitted interleaved with wave 2 below
            # so chunks 2/3 don't eat DVE time ahead of chunk 0/1 tails
            def emit_w1d(g):
                th = talls[g][:, 0:CF]
                sg1 = talls[g][:, CF:2 * CF]
                sg2 = talls[g][:, 2 * CF:3 * CF]

                dD = lg.tile([P, CF], f16, tag="dD")
                v.tensor_scalar(dD[:], th, steer, None, A.mult)
                dc = sh.tile([P, CF], f16, tag="dc")
                v.tensor_scalar(dc[:], dD[:], 0.75, -0.75, A.min, A.max)
                fbD = lg.tile([P, CF], f16, tag="fbD")
                v.tensor_scalar(fbD[:], sg1, hbrake, hbrake, A.mult, A.add)
                fxD = lg.tile([P, CF], f16, tag="fxD")
                v.tensor_scalar(fxD[:], sg2, hthr, hthr, A.mult, A.add)
                fb65 = w1p.tile([P, CF], f16, tag="fb65")
                v.tensor_scalar(fb65[:], fbD[:], 6.5, None, A.mult)
                w1t = w1p.tile([P, CF], f16, tag="w1t")
                v.tensor_scalar(w1t[:], fxD[:], 2.8, None, A.mult)
                t3 = w1p.tile([P, CF], f16, tag="t3")
                v.tensor_tensor(t3[:], w1t[:], fb65[:], A.subtract)
                apg = ap3[:, g * NG:(g + 1) * NG, :]
                v.tensor_scalar(apg[:, :, 1:], r3(t3), lo75, None, A.max)
                v.tensor_scalar(apg[:, :, 1:2], apg[:, :, 1:2], s0x10,
                                None, A.add)
                st[g].update(dD=dD, dc=dc, fbD=fbD, fxD=fxD)

            # ---------- wave 2: chunk-sequential ----------
            def emit_chunk(g):
                s = st[g]
                stg = sp.tile([P, CHW], f32, tag="stg")
                s4 = stg[:].rearrange("p (n t c) -> p n t c", n=NG, t=HP1)

                # Pool terminals that only need wave 1 - emitted first so
                # they fill idle Pool time (and claim the staging buffer)
                gp.tensor_scalar(s4[:, :, 1:, 9], r3(s["dD"]), 1.0, None,
                                 A.mult)
                gp.tensor_scalar(s4[:, :, 1:, 10], r3(s["fbD"]), 1.0, None,
                                 A.mult)
                gp.tensor_scalar(s4[:, :, 1:, 11], r3(s["fxD"]), 1.0, None,
                                 A.mult)
                x0b = x0s[:, None, 0:12].broadcast_to([P, NG, 12])
                gp.tensor_scalar(s4[:, :, 0, :], x0b, 1.0, None, A.mult)

                # DVE speed scan + trig chain
                v.tensor_tensor_scan(s2pad[:, g * CFP:(g + 1) * CFP],
                                     apad[:, g * CFP:(g + 1) * CFP],
                                     zeros[:], 0.0, A.add, A.max)
                s2v = s2p3[:, g * NG:(g + 1) * NG, 1:]

                # cosd first (rc needs only it); ordered so chunk 0's sins
                # follow the last tanh immediately, and later chunks' sins
                # queue behind chunk 0's arctan (keeping the one table load
                # as early as possible in the ACT stream)
                cosd = sh.tile([P, CF], f32, tag="cosd")
                i_cd = sc.activation(cosd[:], s["dc"][:], AF.Sin, bias=hpiv)
                sind = sh.tile([P, CF], f16, tag="sind")
                i_sd = sc.activation(sind[:], s["dc"][:], AF.Sin)
                gate = last_w1_act if first_bt[0] is None else first_bt[0]
                add_dep_helper(i_cd.ins, gate.ins, reason="act order")
                add_dep_helper(i_sd.ins, gate.ins, reason="act order")
                rc = sh.tile([P, CF], f32, tag="rc")
                v.reciprocal_approx_fast(rc[:], cosd[:])
                t45c = sh.tile([P, CF], f16, tag="t45c")
                v.scalar_tensor_tensor(t45c[:], sind[:], 0.45 / 12.6,
                                       rc[:], A.mult, A.mult)

                # yaw chain
                mch = sh.tile([P, CF], f32, tag="mch")
                gp.tensor_scalar(r3(mch), s2v, 0.1, 2.0, A.mult, A.max)
                imc = sh.tile([P, CF], f32, tag="imc")
                v.reciprocal_approx_fast(imc[:], mch[:])
                ylc = sh.tile([P, CF], f16, tag="ylc")
                gp.tensor_scalar(ylc[:], imc[:], f981, 0.15, A.mult, A.max)
                rawc = sh.tile([P, CF], f16, tag="rawc")
                v.tensor_tensor(r3(rawc), s2v, r3(t45c), A.mult)
                clpc = sh.tile([P, CF], f16, tag="clpc")
                gp.tensor_scalar(clpc[:], rawc[:], 1.0, -1.0, A.min, A.max)
                yawD = lg2.tile([P, CF], f16, tag="yawD")
                v.tensor_tensor(yawD[:], clpc[:], ylc[:], A.mult)
                yawP = sh.tile([P, CF], f16, tag="yawP")
                v.tensor_scalar(yawP[:], yawD[:], 1.0, None, A.mult)
                yp3 = r3(yawP)
                v.tensor_scalar(yp3[:, :, 0:1], yp3[:, :, 0:1], psi010,
                                None, A.add)

                # heading cumsum, psi, beta, q, wraps, sin/cos
                Pp = lg2.tile([P, CF], f16, tag="Pp")
                v.tensor_tensor_scan(Pp[:], maskc[:], yawP[:], 0.0,
                                     A.mult, A.add)
                sc.activation(s4[:, :, 1:, 2], r3(Pp), AF.Identity, scale=DT)
                btD = lg2.tile([P, CF], f16, tag="btD")
                i_bt = sc.activation(btD[:], t45c[:], AF.Arctan, scale=12.6)
                add_dep_helper(i_bt.ins, last_w1_act.ins,
                               reason="act table order tanh->trig")
                if first_bt[0] is None:
                    first_bt[0] = i_bt
                # chunk 0's post-beta chain gates the first output DMA:
                # boost it (only this tail - boosting the pre-beta part
                # starves wave 1 of later chunks and delays the barrier)
                import contextlib
                boost = tc.high_priority() if g == 0 else contextlib.nullcontext()
                with boost:
                    _emit_tail(g, s, s4, stg, s2v, Pp, btD, yawD)

            def _emit_tail(g, s, s4, stg, s2v, Pp, btD, yawD):
                q = sh.tile([P, CF], f32, tag="q")
                v.scalar_tensor_tensor(q[:], Pp[:], DT, btD[:],
                                       A.mult, A.add)
                # sin path first: the y-chain (vy -> Fy -> py / ay) is the
                # longest terminal tail
                wsc = sh.tile([P, 2 * CF], f32, tag="wsc")
                v.add_range_wrap(wsc[:, 0:CF], q[:], 0.0, PI, 2 * PI)
                snc = sh.tile([P, 2 * CF], f16, tag="snc")
                sc.activation(snc[:, 0:CF], wsc[:, 0:CF], AF.Sin, scale=SINSC)
                v.add_range_wrap(wsc[:, CF:2 * CF], q[:], PI / 2, PI, 2 * PI)
                sc.activation(snc[:, CF:2 * CF], wsc[:, CF:2 * CF], AF.Sin,
                              scale=SINSC)
                sinA = snc[:, 0:CF]
                cosA = snc[:, CF:2 * CF]

                # velocities (x10) + position cumsums (y first: the Pool ay
                # diff is the longest terminal, start it earliest)
                vyD = lg2.tile([P, CF], f16, tag="vyD")
                v.tensor_tensor(r3(vyD), s2v,
                                sinA.rearrange("p (n h) -> p n h", n=NG),
                                A.mult)
                vxD = lg2.tile([P, CF], f16, tag="vxD")
                v.tensor_tensor(r3(vxD), s2v,
                                cosA.rearrange("p (n h) -> p n h", n=NG),
                                A.mult)
                Fx = lg2.tile([P, CF], f16, tag="Fx")
                v.tensor_tensor_scan(Fx[:], maskc[:], vxD[:], 0.0,
                                     A.mult, A.add)
                Fy = lg2.tile([P, CF], f16, tag="Fy")
                v.tensor_tensor_scan(Fy[:], maskc[:], vyD[:], 0.0,
                                     A.mult, A.add)

                # terminals
                vxD3, vyD3 = r3(vxD), r3(vyD)
                sc.activation(s4[:, :, 1:, 8], r3(btD), AF.Copy)
                sc.activation(s4[:, :, 1:, 3], vxD3, AF.Copy, scale=0.1)
                sc.activation(s4[:, :, 1:, 0], r3(Fx), AF.Identity,
                              bias=px0, scale=DT * 0.1)
                sc.activation(s4[:, :, 1:, 1], r3(Fy), AF.Identity,
                              bias=py0, scale=DT * 0.1)
                v.tensor_tensor(s4[:, :, 2:, 6], vxD3[:, :, 1:],
                                vxD3[:, :, :H - 1], A.subtract)
                v.tensor_scalar(s4[:, :, 1, 6], vxD3[:, :, 0], 1.0, vx010,
                                A.mult, A.subtract)
                gp.tensor_scalar(s4[:, :, 1:, 5], r3(yawD), 1.0, None,
                                 A.mult)
                gp.tensor_scalar(s4[:, :, 1:, 4], vyD3, 0.1, None, A.mult)
                gp.tensor_tensor(s4[:, :, 2:, 7], vyD3[:, :, 1:],
                                 vyD3[:, :, :H - 1], A.subtract)
                gp.tensor_scalar(s4[:, :, 1, 7], vyD3[:, :, 0], 1.0, vy010,
                                 A.mult, A.subtract)

                sy.dma_start(out=out_d[:, g * CHW:(g + 1) * CHW], in_=stg[:])

            # emission order: chunks 0/1 (and their wave-1 DVE clusters)
            # first so the DVE stream serves the first two output DMAs
            # before chunks 2/3 consume any DVE time
            emit_w1d(0)
            emit_w1d(1)
            with tc.high_priority():
                emit_chunk(0)
            emit_chunk(1)
            emit_w1d(2)
            emit_chunk(2)
            emit_w1d(3)
            emit_chunk(3)

    bacc.get_activation_tables = _patched_tables
    try:
        nc.compile()
    finally:
        bacc.get_activation_tables = _orig_tables
    return nc


def _get_built():
    global _BUILT
    if _BUILT is None:
        _BUILT = _build_kernel()
    return _BUILT


def _run(x0, controls, deg, trace=False):
    from concourse.bass_utils import run_bass_kernel_spmd

    x0 = np.ascontiguousarray(x0, dtype=np.float32)
    controls = np.ascontiguousarray(controls, dtype=np.float32)
    deg = np.ascontiguousarray(deg, dtype=np.float32)

    nc = _get_built()
    in_maps = []
    for c in range(NCORES):
        sl = slice(c * BC, (c + 1) * BC)
        # [P, NPT, H, 3] -> per chunk channel-major [P, G, 3, NG, H];
        # u1/u2 prescaled by 0.5 (tanh half-angle sigmoid) so wave 1 is a
        # single full-scale tanh per chunk
        ctrl_c = controls[sl].reshape(R, H, 3).reshape(P, G, NG, H, 3)
        ctrl_c = ctrl_c.transpose(0, 1, 4, 2, 3).copy()   # [P, G, 3, NG, H]
        ctrl_c[:, :, 1:3] *= 0.5
        ctrl_c = ctrl_c.reshape(P, NPT * H * 3)
        x0p = np.repeat(x0[sl], P // BC, axis=0)      # [128, 12]
        degp = np.repeat(deg[sl], P // BC, axis=0)    # [128, 5]
        x0f = x0p.astype(np.float64)
        scal = np.zeros((P, 16), dtype=np.float32)
        fric = np.maximum(degp[:, 4], 0.1)
        scal[:, 0] = np.maximum(degp[:, 0], 0.05)            # steer
        scal[:, 3] = -7.5 * fric                             # lo (x10)
        scal[:, 4] = 9.81 * fric                             # f981
        scal[:, 5] = 10.0 * np.sqrt(x0f[:, 3] ** 2 + x0f[:, 4] ** 2 + 1e-6)
        scal[:, 7] = x0p[:, 0]                               # px0
        scal[:, 8] = x0p[:, 1]                               # py0
        scal[:, 9] = 10.0 * x0p[:, 3]                        # 10*vx0
        scal[:, 10] = 10.0 * x0p[:, 4]                       # 10*vy0
        scal[:, 11] = x0p[:, 2] / DT                         # psi0/DT
        scal[:, 12] = 0.5 * np.maximum(degp[:, 1], 0.05)     # brake/2
        scal[:, 13] = 0.5 * np.maximum(degp[:, 2], 0.05)     # thr/2
        scal[:, 14] = np.pi / 2
        in_maps.append({
            "ctrl": np.ascontiguousarray(ctrl_c.astype(np.float16)),
            "x0p": np.ascontiguousarray(x0p),
            "scal": scal,
        })

    res = run_bass_kernel_spmd(nc, in_maps, list(range(NCORES)), trace=trace)
    outs = []
    for c in range(NCORES):
        o = np.asarray(res.results[c]["out"])
        outs.append(o.reshape(R, HP1, CW).reshape(BC, L, HP1, CW))
    return np.concatenate(outs, axis=0), res


def kernel(x0: np.ndarray, controls: np.ndarray, deg: np.ndarray) -> np.ndarray:
    out, _ = _run(x0, controls, deg)
    return out


if __name__ == "__main__":
    rng = np.random.default_rng(0)
    x0 = rng.standard_normal((B, 12)).astype(np.float32)
    controls = rng.standard_normal((B, L, H, 3)).astype(np.float32)
    deg = rng.random((B, 5)).astype(np.float32)
    out = kernel(x0, controls, deg)
    print("out", out.shape, out.dtype)


# revision 53
# speedup vs baseline: 1.0060x; 1.0060x over previous
"""Degraded bicycle rollout kernel for Trainium2 (8 NeuronCores, data-parallel on batch).

Structure (v4):
  - Speed recurrence s' = max(s + a, 0) (eps-free, x10-scaled) solved by ONE
    hardware tensor_tensor_scan per chunk over an 81-slot padded layout:
    slot 0 of each rollout holds -60000 (clamps the running state to 0 =
    segment reset), slot 1 is pre-biased with 10*s0. psi0 is folded into
    the yaw cumsum the same way (first-slot bias on a scratch copy).
  - Controls DMA'd as fp16 channel-major per chunk, so wave 1 is two ACT
    ops (tanh(u0), tanh(0.5*[u1 u2])); sigmoid comes from the half-angle
    identity folded into the following TSP scalars.
  - Wave 1 is Tanh-only and shares the silu_and_others ACT table with Sin;
    only the Arctan switches tables (2 LoadActFuncSet total). All wave-1
    ACT runs before wave 2 (the arctan dep enforces it), so the first
    chunk's trig chain starts as early as possible.
  - Wave 2 is emitted strictly chunk-sequentially (engines are in-order;
    clustering a chunk's chain finishes chunk 0 - and its output DMA -
    first). All navigation state lives in dense tiles; staging writes are
    terminal copies, so a chunk blocked on a staging buffer still runs its
    full compute chain during earlier output DMAs.
  - fp16 intermediates give DVE 2x/4x perf modes where supported.

Layout per core: 4096 rollouts = 128 partitions x 32; partition p holds
rollouts p*32..p*32+31 of batch p//2. Free dim rollout-major f = n*80 + t.
"""

import sys

sys.path.insert(0, "/opt/trn_rl_repo")

import numpy as np

B, L, H = 512, 64, 80
NCORES = 8
BC = B // NCORES          # 64 batches per core
R = BC * L                # 4096 rollouts per core
P = 128
NPT = R // P              # 32 rollouts per partition
F = NPT * H               # 2560 elements per partition
HP1 = H + 1
CW = 12                   # output channels
SW = HP1 * CW             # 972 staging words per rollout
G = 4                     # chunks
NG = NPT // G             # 8 rollouts per partition per chunk
CF = NG * H               # 640 free elems per chunk
CFP = NG * (H + 1)        # 648 padded elems per chunk
CHW = NG * SW             # 7776 staging words per partition per chunk
DT = 0.1
WB = 2.8
PI = float(np.pi)
PAD = -60000.0            # fp16-safe segment-reset value
SINSC = 0.999999          # keeps wrapped args strictly inside [-pi, pi]

_BUILT = None


def _build_kernel():
    import concourse.bass as bass
    import concourse.bacc as bacc
    import concourse.mybir as mybir
    from concourse.tile import TileContext
    from concourse.tile_rust import add_dep_helper

    f32 = mybir.dt.float32
    f16 = mybir.dt.float16
    A = mybir.AluOpType
    AF = mybir.ActivationFunctionType

    # The act-table insertion pass is first-fit over act_info.json order.
    # Our op mix (tanh/sin/arctan/identity) is covered 2-load by
    # silu_and_others (via the dummy Silu below) + trig_and_small, but
    # first-fit would send the first Arctan to sigmoid_and_others (earlier
    # index, lacks sin) and force a third load. Hide Arctan from that set
    # during this build only - indices (= emitted act_func_set_ids) are
    # unchanged, so the compiled program stays fully valid.
    import concourse.hw_specs as _hws

    _orig_tables = _hws.get_activation_tables

    def _patched_tables(arch):
        d = dict(_orig_tables(arch))
        s = set(d.get("sigmoid_and_others", set()))
        s.discard(mybir.ActivationFunctionType.Arctan)
        d["sigmoid_and_others"] = s
        return d

    nc = bacc.Bacc(None, target_bir_lowering=False)
    ctrl_d = nc.declare_dram_parameter("ctrl", [P, NPT * H * 3], f16, isOutput=False)
    x0_d = nc.declare_dram_parameter("x0p", [P, 12], f32, isOutput=False)
    scal_d = nc.declare_dram_parameter("scal", [P, 16], f32, isOutput=False)
    out_d = nc.declare_dram_parameter("out", [P, NPT * SW], f32, isOutput=True)

    with TileContext(nc) as tc:
        v = nc.vector
        sc = nc.scalar
        gp = nc.gpsimd
        sy = nc.sync

        with tc.tile_pool(name="pers", bufs=1) as pp, \
             tc.tile_pool(name="ctrlp", bufs=4) as ctp, \
             tc.tile_pool(name="w1a", bufs=2) as w1p, \
             tc.tile_pool(name="lng", bufs=4) as lg, \
             tc.tile_pool(name="lg2", bufs=2) as lg2, \
             tc.tile_pool(name="sht", bufs=2) as sh, \
             tc.tile_pool(name="stgp", bufs=2) as sp:

            # ---------- persistent scalars / constants ----------
            x0s = pp.tile([P, 12], f32, tag="x0s")
            scal = pp.tile([P, 16], f32, tag="scal")
            steer = scal[:, 0:1]
            lo75 = scal[:, 3:4]      # -7.5*friction  (x10 accDT bound)
            f981 = scal[:, 4:5]      # 9.81*friction
            s0x10 = scal[:, 5:6]     # 10*sqrt(vx0^2+vy0^2+1e-6)
            px0 = scal[:, 7:8]
            py0 = scal[:, 8:9]
            vx010 = scal[:, 9:10]    # 10*vx0
            vy010 = scal[:, 10:11]   # 10*vy0
            psi010 = scal[:, 11:12]  # psi0/DT
            hbrake = scal[:, 12:13]  # 0.5*brake
            hthr = scal[:, 13:14]    # 0.5*thr
            hpiv = scal[:, 14:15]    # pi/2

            # ---------- input DMAs: ctrl chunk 0 first (critical path) ----
            CTW = NG * H * 3
            ctiles = []
            ct0 = ctp.tile([P, CTW], f16, tag="ctrl")
            sy.dma_start(out=ct0[:], in_=ctrl_d[:, 0:CTW])
            ctiles.append(ct0)
            sy.dma_start(out=x0s[:], in_=x0_d[:])
            sy.dma_start(out=scal[:], in_=scal_d[:])
            for g in range(1, G):
                ct = ctp.tile([P, CTW], f16, tag="ctrl")
                sy.dma_start(out=ct[:], in_=ctrl_d[:, g * CTW:(g + 1) * CTW])
                ctiles.append(ct)

            # Dummy Silu: the table pass is first-fit, so the first ACT op
            # determines the loaded set. Silu's first-containing set is
            # silu_and_others = {silu, tanh, sin, identity, copy, ...} which
            # covers everything here except Arctan -> the whole sin chain
            # runs without waiting for a table switch.
            dmy = pp.tile([P, 1], f32, tag="dmy")
            v.memset(dmy[:], 0.0)
            sc.activation(dmy[:], dmy[:], AF.Silu)

            maskc = pp.tile([P, CF], f16, tag="maskc")
            v.memset(maskc[:], 1.0)
            mc3 = maskc[:].rearrange("p (n h) -> p n h", n=NG)
            v.memset(mc3[:, :, 0:1], 0.0)

            zeros = pp.tile([P, CFP], f16, tag="zeros")
            v.memset(zeros[:], 0.0)

            apad = pp.tile([P, NPT * (H + 1)], f16, tag="apad")
            ap3 = apad[:].rearrange("p (n h) -> p n h", n=NPT)
            v.memset(ap3[:, :, 0:1], PAD)
            s2pad = pp.tile([P, NPT * (H + 1)], f16, tag="s2pad")
            s2p3 = s2pad[:].rearrange("p (n h) -> p n h", n=NPT)

            st = [dict() for _ in range(G)]
            first_bt = [None]

            def r3(t):
                return t[:].rearrange("p (n h) -> p n h", n=NG)

            # ---------- wave 1 ACT: all chunks (Tanh-only phase; u1/u2 are
            # host-prescaled by 0.5 so each chunk is ONE tanh op) ----------
            last_w1_act = None
            talls = []
            for g in range(G):
                tall = w1p.tile([P, 3 * CF], f16, tag="tall", bufs=4)
                i = sc.activation(tall[:], ctiles[g][:], AF.Tanh)
                last_w1_act = i
                talls.append(tall)

            # wave-1 DVE clusters are emitted interleaved with wave 2 below
            # so chunks 2/3 don't eat DVE time ahead of chunk 0/1 tails
            def emit_w1d(g):
                th = talls[g][:, 0:CF]
                sg1 = talls[g][:, CF:2 * CF]
                sg2 = talls[g][:, 2 * CF:3 * CF]

                dD = lg.tile([P, CF], f16, tag="dD")
                v.tensor_scalar(dD[:], th, steer, None, A.mult)
                dc = sh.tile([P, CF], f16, tag="dc")
                v.tensor_scalar(dc[:], dD[:], 0.75, -0.75, A.min, A.max)
                fbD = lg.tile([P, CF], f16, tag="fbD")
                v.tensor_scalar(fbD[:], sg1, hbrake, hbrake, A.mult, A.add)
                fxD = lg.tile([P, CF], f16, tag="fxD")
                v.tensor_scalar(fxD[:], sg2, hthr, hthr, A.mult, A.add)
                fb65 = w1p.tile([P, CF], f16, tag="fb65")
                v.tensor_scalar(fb65[:], fbD[:], 6.5, None, A.mult)
                w1t = w1p.tile([P, CF], f16, tag="w1t")
                v.tensor_scalar(w1t[:], fxD[:], 2.8, None, A.mult)
                t3 = w1p.tile([P, CF], f16, tag="t3")
                v.tensor_tensor(t3[:], w1t[:], fb65[:], A.subtract)
                apg = ap3[:, g * NG:(g + 1) * NG, :]
                v.tensor_scalar(apg[:, :, 1:], r3(t3), lo75, None, A.max)
                v.tensor_scalar(apg[:, :, 1:2], apg[:, :, 1:2], s0x10,
                                None, A.add)
                st[g].update(dD=dD, dc=dc, fbD=fbD, fxD=fxD)

            # ---------- wave 2: chunk-sequential ----------
            def emit_chunk(g):
                s = st[g]
                stg = sp.tile([P, CHW], f32, tag="stg")
                s4 = stg[:].rearrange("p (n t c) -> p n t c", n=NG, t=HP1)

                # Pool terminals that only need wave 1 - emitted first so
                # they fill idle Pool time (and claim the staging buffer)
                gp.tensor_scalar(s4[:, :, 1:, 9], r3(s["dD"]), 1.0, None,
                                 A.mult)
                gp.tensor_scalar(s4[:, :, 1:, 10], r3(s["fbD"]), 1.0, None,
                                 A.mult)
                gp.tensor_scalar(s4[:, :, 1:, 11], r3(s["fxD"]), 1.0, None,
                                 A.mult)
                x0b = x0s[:, None, 0:12].broadcast_to([P, NG, 12])
                gp.tensor_scalar(s4[:, :, 0, :], x0b, 1.0, None, A.mult)

                # DVE speed scan + trig chain
                v.tensor_tensor_scan(s2pad[:, g * CFP:(g + 1) * CFP],
                                     apad[:, g * CFP:(g + 1) * CFP],
                                     zeros[:], 0.0, A.add, A.max)
                s2v = s2p3[:, g * NG:(g + 1) * NG, 1:]

                # cosd first (rc needs only it); ordered so chunk 0's sins
                # follow the last tanh immediately, and later chunks' sins
                # queue behind chunk 0's arctan (keeping the one table load
                # as early as possible in the ACT stream)
                cosd = sh.tile([P, CF], f32, tag="cosd")
                i_cd = sc.activation(cosd[:], s["dc"][:], AF.Sin, bias=hpiv)
                sind = sh.tile([P, CF], f16, tag="sind")
                i_sd = sc.activation(sind[:], s["dc"][:], AF.Sin)
                add_dep_helper(i_cd.ins, last_w1_act.ins, reason="act order")
                add_dep_helper(i_sd.ins, last_w1_act.ins, reason="act order")
                if g == 0:
                    # tiny arctan right after chunk 0's sins: the first-fit
                    # pass hangs the trig table load on it, so the switch
                    # overlaps the rc/t45c DVE work instead of sitting on
                    # the beta critical path
                    i_dm2 = sc.activation(dmy[:], dmy[:], AF.Arctan)
                    add_dep_helper(i_dm2.ins, i_sd.ins, reason="load hoist")
                    add_dep_helper(i_dm2.ins, i_cd.ins, reason="load hoist")
                rc = sh.tile([P, CF], f32, tag="rc")
                v.reciprocal_approx_fast(rc[:], cosd[:])
                t45c = sh.tile([P, CF], f16, tag="t45c")
                v.scalar_tensor_tensor(t45c[:], sind[:], 0.45 / 12.6,
                                       rc[:], A.mult, A.mult)

                # yaw chain
                mch = sh.tile([P, CF], f32, tag="mch")
                gp.tensor_scalar(r3(mch), s2v, 0.1, 2.0, A.mult, A.max)
                imc = sh.tile([P, CF], f32, tag="imc")
                v.reciprocal_approx_fast(imc[:], mch[:])
                ylc = sh.tile([P, CF], f16, tag="ylc")
                gp.tensor_scalar(ylc[:], imc[:], f981, 0.15, A.mult, A.max)
                rawc = sh.tile([P, CF], f16, tag="rawc")
                v.tensor_tensor(r3(rawc), s2v, r3(t45c), A.mult)
                clpc = sh.tile([P, CF], f16, tag="clpc")
                gp.tensor_scalar(clpc[:], rawc[:], 1.0, -1.0, A.min, A.max)
                yawD = lg2.tile([P, CF], f16, tag="yawD")
                v.tensor_tensor(yawD[:], clpc[:], ylc[:], A.mult)
                yawP = sh.tile([P, CF], f16, tag="yawP")
                v.tensor_scalar(yawP[:], yawD[:], 1.0, None, A.mult)
                yp3 = r3(yawP)
                v.tensor_scalar(yp3[:, :, 0:1], yp3[:, :, 0:1], psi010,
                                None, A.add)

                # heading cumsum, psi, beta, q, wraps, sin/cos
                Pp = lg2.tile([P, CF], f16, tag="Pp")
                v.tensor_tensor_scan(Pp[:], maskc[:], yawP[:], 0.0,
                                     A.mult, A.add)
                sc.activation(s4[:, :, 1:, 2], r3(Pp), AF.Identity, scale=DT)
                btD = lg2.tile([P, CF], f16, tag="btD")
                i_bt = sc.activation(btD[:], t45c[:], AF.Arctan, scale=12.6)
                add_dep_helper(i_bt.ins, last_w1_act.ins,
                               reason="act table order tanh->trig")
                if first_bt[0] is None:
                    first_bt[0] = i_bt
                # chunk 0's post-beta chain gates the first output DMA:
                # boost it (only this tail - boosting the pre-beta part
                # starves wave 1 of later chunks and delays the barrier)
                import contextlib
                boost = tc.high_priority() if g == 0 else contextlib.nullcontext()
                with boost:
                    _emit_tail(g, s, s4, stg, s2v, Pp, btD, yawD)

            def _emit_tail(g, s, s4, stg, s2v, Pp, btD, yawD):
                q = sh.tile([P, CF], f32, tag="q")
                v.scalar_tensor_tensor(q[:], Pp[:], DT, btD[:],
                                       A.mult, A.add)
                # sin path first: the y-chain (vy -> Fy -> py / ay) is the
                # longest terminal tail
                wsc = sh.tile([P, 2 * CF], f32, tag="wsc")
                v.add_range_wrap(wsc[:, 0:CF], q[:], 0.0, PI, 2 * PI)
                snc = sh.tile([P, 2 * CF], f16, tag="snc")
                sc.activation(snc[:, 0:CF], wsc[:, 0:CF], AF.Sin, scale=SINSC)
                v.add_range_wrap(wsc[:, CF:2 * CF], q[:], PI / 2, PI, 2 * PI)
                sc.activation(snc[:, CF:2 * CF], wsc[:, CF:2 * CF], AF.Sin,
                              scale=SINSC)
                sinA = snc[:, 0:CF]
                cosA = snc[:, CF:2 * CF]

                # velocities (x10) + position cumsums (y first: the Pool ay
                # diff is the longest terminal, start it earliest)
                vyD = lg2.tile([P, CF], f16, tag="vyD")
                v.tensor_tensor(r3(vyD), s2v,
                                sinA.rearrange("p (n h) -> p n h", n=NG),
                                A.mult)
                vxD = lg2.tile([P, CF], f16, tag="vxD")
                v.tensor_tensor(r3(vxD), s2v,
                                cosA.rearrange("p (n h) -> p n h", n=NG),
                                A.mult)
                Fx = lg2.tile([P, CF], f16, tag="Fx")
                v.tensor_tensor_scan(Fx[:], maskc[:], vxD[:], 0.0,
                                     A.mult, A.add)
                Fy = lg2.tile([P, CF], f16, tag="Fy")
                v.tensor_tensor_scan(Fy[:], maskc[:], vyD[:], 0.0,
                                     A.mult, A.add)

                # terminals
                vxD3, vyD3 = r3(vxD), r3(vyD)
                sc.activation(s4[:, :, 1:, 8], r3(btD), AF.Copy)
                sc.activation(s4[:, :, 1:, 3], vxD3, AF.Copy, scale=0.1)
                sc.activation(s4[:, :, 1:, 0], r3(Fx), AF.Identity,
                              bias=px0, scale=DT * 0.1)
                sc.activation(s4[:, :, 1:, 1], r3(Fy), AF.Identity,
                              bias=py0, scale=DT * 0.1)
                v.tensor_tensor(s4[:, :, 2:, 6], vxD3[:, :, 1:],
                                vxD3[:, :, :H - 1], A.subtract)
                v.tensor_scalar(s4[:, :, 1, 6], vxD3[:, :, 0], 1.0, vx010,
                                A.mult, A.subtract)
                gp.tensor_scalar(s4[:, :, 1:, 5], r3(yawD), 1.0, None,
                                 A.mult)
                gp.tensor_scalar(s4[:, :, 1:, 4], vyD3, 0.1, None, A.mult)
                gp.tensor_tensor(s4[:, :, 2:, 7], vyD3[:, :, 1:],
                                 vyD3[:, :, :H - 1], A.subtract)
                gp.tensor_scalar(s4[:, :, 1, 7], vyD3[:, :, 0], 1.0, vy010,
                                 A.mult, A.subtract)

                sy.dma_start(out=out_d[:, g * CHW:(g + 1) * CHW], in_=stg[:])

            # emission order: chunks 0/1 (and their wave-1 DVE clusters)
            # first so the DVE stream serves the first two output DMAs
            # before chunks 2/3 consume any DVE time
            emit_w1d(0)
            emit_w1d(1)
            with tc.high_priority():
                emit_chunk(0)
            emit_chunk(1)
            emit_w1d(2)
            emit_chunk(2)
            emit_w1d(3)
            emit_chunk(3)

    bacc.get_activation_tables = _patched_tables
    try:
        nc.compile()
    finally:
        bacc.get_activation_tables = _orig_tables
    return nc


def _get_built():
    global _BUILT
    if _BUILT is None:
        _BUILT = _build_kernel()
    return _BUILT


def _run(x0, controls, deg, trace=False):
    from concourse.bass_utils import run_bass_kernel_spmd

    x0 = np.ascontiguousarray(x0, dtype=np.float32)
    controls = np.ascontiguousarray(controls, dtype=np.float32)
    deg = np.ascontiguousarray(deg, dtype=np.float32)

    nc = _get_built()
    in_maps = []
    for c in range(NCORES):
        sl = slice(c * BC, (c + 1) * BC)
        # [P, NPT, H, 3] -> per chunk channel-major [P, G, 3, NG, H];
        # u1/u2 prescaled by 0.5 (tanh half-angle sigmoid) so wave 1 is a
        # single full-scale tanh per chunk
        ctrl_c = controls[sl].reshape(R, H, 3).reshape(P, G, NG, H, 3)
        ctrl_c = ctrl_c.transpose(0, 1, 4, 2, 3).copy()   # [P, G, 3, NG, H]
        ctrl_c[:, :, 1:3] *= 0.5
        ctrl_c = ctrl_c.reshape(P, NPT * H * 3)
        x0p = np.repeat(x0[sl], P // BC, axis=0)      # [128, 12]
        degp = np.repeat(deg[sl], P // BC, axis=0)    # [128, 5]
        x0f = x0p.astype(np.float64)
        scal = np.zeros((P, 16), dtype=np.float32)
        fric = np.maximum(degp[:, 4], 0.1)
        scal[:, 0] = np.maximum(degp[:, 0], 0.05)            # steer
        scal[:, 3] = -7.5 * fric                             # lo (x10)
        scal[:, 4] = 9.81 * fric                             # f981
        scal[:, 5] = 10.0 * np.sqrt(x0f[:, 3] ** 2 + x0f[:, 4] ** 2 + 1e-6)
        scal[:, 7] = x0p[:, 0]                               # px0
        scal[:, 8] = x0p[:, 1]                               # py0
        scal[:, 9] = 10.0 * x0p[:, 3]                        # 10*vx0
        scal[:, 10] = 10.0 * x0p[:, 4]                       # 10*vy0
        scal[:, 11] = x0p[:, 2] / DT                         # psi0/DT
        scal[:, 12] = 0.5 * np.maximum(degp[:, 1], 0.05)     # brake/2
        scal[:, 13] = 0.5 * np.maximum(degp[:, 2], 0.05)     # thr/2
        scal[:, 14] = np.pi / 2
        in_maps.append({
            "ctrl": np.ascontiguousarray(ctrl_c.astype(np.float16)),
            "x0p": np.ascontiguousarray(x0p),
            "scal": scal,
        })

    res = run_bass_kernel_spmd(nc, in_maps, list(range(NCORES)), trace=trace)
    outs = []
    for c in range(NCORES):
        o = np.asarray(res.results[c]["out"])
        outs.append(o.reshape(R, HP1, CW).reshape(BC, L, HP1, CW))
    return np.concatenate(outs, axis=0), res


def kernel(x0: np.ndarray, controls: np.ndarray, deg: np.ndarray) -> np.ndarray:
    out, _ = _run(x0, controls, deg)
    return out


if __name__ == "__main__":
    rng = np.random.default_rng(0)
    x0 = rng.standard_normal((B, 12)).astype(np.float32)
    controls = rng.standard_normal((B, L, H, 3)).astype(np.float32)
    deg = rng.random((B, 5)).astype(np.float32)
    out = kernel(x0, controls, deg)
    print("out", out.shape, out.dtype)


# revision 54
# speedup vs baseline: 1.0161x; 1.0100x over previous
"""Degraded bicycle rollout kernel for Trainium2 (8 NeuronCores, data-parallel on batch).

Structure (v4):
  - Speed recurrence s' = max(s + a, 0) (eps-free, x10-scaled) solved by ONE
    hardware tensor_tensor_scan per chunk over an 81-slot padded layout:
    slot 0 of each rollout holds -60000 (clamps the running state to 0 =
    segment reset), slot 1 is pre-biased with 10*s0. psi0 is folded into
    the yaw cumsum the same way (first-slot bias on a scratch copy).
  - Controls DMA'd as fp16 channel-major per chunk, so wave 1 is two ACT
    ops (tanh(u0), tanh(0.5*[u1 u2])); sigmoid comes from the half-angle
    identity folded into the following TSP scalars.
  - Wave 1 is Tanh-only and shares the silu_and_others ACT table with Sin;
    only the Arctan switches tables (2 LoadActFuncSet total). All wave-1
    ACT runs before wave 2 (the arctan dep enforces it), so the first
    chunk's trig chain starts as early as possible.
  - Wave 2 is emitted strictly chunk-sequentially (engines are in-order;
    clustering a chunk's chain finishes chunk 0 - and its output DMA -
    first). All navigation state lives in dense tiles; staging writes are
    terminal copies, so a chunk blocked on a staging buffer still runs its
    full compute chain during earlier output DMAs.
  - fp16 intermediates give DVE 2x/4x perf modes where supported.

Layout per core: 4096 rollouts = 128 partitions x 32; partition p holds
rollouts p*32..p*32+31 of batch p//2. Free dim rollout-major f = n*80 + t.
"""

import sys

sys.path.insert(0, "/opt/trn_rl_repo")

import numpy as np

B, L, H = 512, 64, 80
NCORES = 8
BC = B // NCORES          # 64 batches per core
R = BC * L                # 4096 rollouts per core
P = 128
NPT = R // P              # 32 rollouts per partition
F = NPT * H               # 2560 elements per partition
HP1 = H + 1
CW = 12                   # output channels
SW = HP1 * CW             # 972 staging words per rollout
G = 4                     # chunks
NG = NPT // G             # 8 rollouts per partition per chunk
CF = NG * H               # 640 free elems per chunk
CFP = NG * (H + 1)        # 648 padded elems per chunk
CHW = NG * SW             # 7776 staging words per partition per chunk
DT = 0.1
WB = 2.8
PI = float(np.pi)
PAD = -60000.0            # fp16-safe segment-reset value
SINSC = 0.999999          # keeps wrapped args strictly inside [-pi, pi]

_BUILT = None


def _build_kernel():
    import concourse.bass as bass
    import concourse.bacc as bacc
    import concourse.mybir as mybir
    from concourse.tile import TileContext
    from concourse.tile_rust import add_dep_helper

    f32 = mybir.dt.float32
    f16 = mybir.dt.float16
    A = mybir.AluOpType
    AF = mybir.ActivationFunctionType

    # The act-table insertion pass is first-fit over act_info.json order.
    # Our op mix (tanh/sin/arctan/identity) is covered 2-load by
    # silu_and_others (via the dummy Silu below) + trig_and_small, but
    # first-fit would send the first Arctan to sigmoid_and_others (earlier
    # index, lacks sin) and force a third load. Hide Arctan from that set
    # during this build only - indices (= emitted act_func_set_ids) are
    # unchanged, so the compiled program stays fully valid.
    import concourse.hw_specs as _hws

    _orig_tables = _hws.get_activation_tables

    def _patched_tables(arch):
        d = dict(_orig_tables(arch))
        s = set(d.get("sigmoid_and_others", set()))
        s.discard(mybir.ActivationFunctionType.Arctan)
        d["sigmoid_and_others"] = s
        return d

    nc = bacc.Bacc(None, target_bir_lowering=False)
    ctrl_d = nc.declare_dram_parameter("ctrl", [P, NPT * H * 3], f16, isOutput=False)
    x0_d = nc.declare_dram_parameter("x0p", [P, 12], f32, isOutput=False)
    scal_d = nc.declare_dram_parameter("scal", [P, 16], f32, isOutput=False)
    out_d = nc.declare_dram_parameter("out", [P, NPT * SW], f32, isOutput=True)

    with TileContext(nc) as tc:
        v = nc.vector
        sc = nc.scalar
        gp = nc.gpsimd
        sy = nc.sync

        with tc.tile_pool(name="pers", bufs=1) as pp, \
             tc.tile_pool(name="ctrlp", bufs=4) as ctp, \
             tc.tile_pool(name="w1a", bufs=2) as w1p, \
             tc.tile_pool(name="lng", bufs=4) as lg, \
             tc.tile_pool(name="lg2", bufs=2) as lg2, \
             tc.tile_pool(name="sht", bufs=2) as sh, \
             tc.tile_pool(name="stgp", bufs=2) as sp:

            # ---------- persistent scalars / constants ----------
            x0s = pp.tile([P, 12], f32, tag="x0s")
            scal = pp.tile([P, 16], f32, tag="scal")
            steer = scal[:, 0:1]
            lo75 = scal[:, 3:4]      # -7.5*friction  (x10 accDT bound)
            f981 = scal[:, 4:5]      # 9.81*friction
            s0x10 = scal[:, 5:6]     # 10*sqrt(vx0^2+vy0^2+1e-6)
            px0 = scal[:, 7:8]
            py0 = scal[:, 8:9]
            vx010 = scal[:, 9:10]    # 10*vx0
            vy010 = scal[:, 10:11]   # 10*vy0
            psi010 = scal[:, 11:12]  # psi0/DT
            hbrake = scal[:, 12:13]  # 0.5*brake
            hthr = scal[:, 13:14]    # 0.5*thr
            hpiv = scal[:, 14:15]    # pi/2

            # ---------- input DMAs: ctrl chunk 0 first (critical path) ----
            CTW = NG * H * 3
            ctiles = []
            ct0 = ctp.tile([P, CTW], f16, tag="ctrl")
            sy.dma_start(out=ct0[:], in_=ctrl_d[:, 0:CTW])
            ctiles.append(ct0)
            sy.dma_start(out=x0s[:], in_=x0_d[:])
            sy.dma_start(out=scal[:], in_=scal_d[:])
            for g in range(1, G):
                ct = ctp.tile([P, CTW], f16, tag="ctrl")
                sy.dma_start(out=ct[:], in_=ctrl_d[:, g * CTW:(g + 1) * CTW])
                ctiles.append(ct)

            # Dummy Silu: the table pass is first-fit, so the first ACT op
            # determines the loaded set. Silu's first-containing set is
            # silu_and_others = {silu, tanh, sin, identity, copy, ...} which
            # covers everything here except Arctan -> the whole sin chain
            # runs without waiting for a table switch.
            dmy = pp.tile([P, 1], f32, tag="dmy")
            v.memset(dmy[:], 0.0)
            sc.activation(dmy[:], dmy[:], AF.Silu)

            maskc = pp.tile([P, CF], f16, tag="maskc")
            v.memset(maskc[:], 1.0)
            mc3 = maskc[:].rearrange("p (n h) -> p n h", n=NG)
            v.memset(mc3[:, :, 0:1], 0.0)

            zeros = pp.tile([P, CFP], f16, tag="zeros")
            v.memset(zeros[:], 0.0)

            apad = pp.tile([P, NPT * (H + 1)], f16, tag="apad")
            ap3 = apad[:].rearrange("p (n h) -> p n h", n=NPT)
            v.memset(ap3[:, :, 0:1], PAD)
            s2pad = pp.tile([P, NPT * (H + 1)], f16, tag="s2pad")
            s2p3 = s2pad[:].rearrange("p (n h) -> p n h", n=NPT)

            st = [dict() for _ in range(G)]
            first_bt = [None]

            def r3(t):
                return t[:].rearrange("p (n h) -> p n h", n=NG)

            # ---------- wave 1 ACT: all chunks (Tanh-only phase; u1/u2 are
            # host-prescaled by 0.5 so each chunk is ONE tanh op) ----------
            last_w1_act = None
            talls = []
            for g in range(G):
                tall = w1p.tile([P, 3 * CF], f16, tag="tall", bufs=4)
                i = sc.activation(tall[:], ctiles[g][:], AF.Tanh)
                last_w1_act = i
                talls.append(tall)

            # wave-1 DVE clusters are emitted interleaved with wave 2 below
            # so chunks 2/3 don't eat DVE time ahead of chunk 0/1 tails
            def emit_w1d(g):
                th = talls[g][:, 0:CF]
                sg1 = talls[g][:, CF:2 * CF]
                sg2 = talls[g][:, 2 * CF:3 * CF]

                dD = lg.tile([P, CF], f16, tag="dD")
                v.tensor_scalar(dD[:], th, steer, None, A.mult)
                dc = sh.tile([P, CF], f16, tag="dc")
                v.tensor_scalar(dc[:], dD[:], 0.75, -0.75, A.min, A.max)
                fbD = lg.tile([P, CF], f16, tag="fbD")
                v.tensor_scalar(fbD[:], sg1, hbrake, hbrake, A.mult, A.add)
                fxD = lg.tile([P, CF], f16, tag="fxD")
                v.tensor_scalar(fxD[:], sg2, hthr, hthr, A.mult, A.add)
                fb65 = w1p.tile([P, CF], f16, tag="fb65")
                v.tensor_scalar(fb65[:], fbD[:], 6.5, None, A.mult)
                w1t = w1p.tile([P, CF], f16, tag="w1t")
                v.tensor_scalar(w1t[:], fxD[:], 2.8, None, A.mult)
                t3 = w1p.tile([P, CF], f16, tag="t3")
                v.tensor_tensor(t3[:], w1t[:], fb65[:], A.subtract)
                apg = ap3[:, g * NG:(g + 1) * NG, :]
                v.tensor_scalar(apg[:, :, 1:], r3(t3), lo75, None, A.max)
                v.tensor_scalar(apg[:, :, 1:2], apg[:, :, 1:2], s0x10,
                                None, A.add)
                st[g].update(dD=dD, dc=dc, fbD=fbD, fxD=fxD)

            # ---------- wave 2: chunk-sequential ----------
            def emit_chunk(g):
                s = st[g]
                stg = sp.tile([P, CHW], f32, tag="stg")
                s4 = stg[:].rearrange("p (n t c) -> p n t c", n=NG, t=HP1)

                # Pool terminals that only need wave 1 - emitted first so
                # they fill idle Pool time (and claim the staging buffer)
                gp.tensor_scalar(s4[:, :, 1:, 9], r3(s["dD"]), 1.0, None,
                                 A.mult)
                gp.tensor_scalar(s4[:, :, 1:, 10], r3(s["fbD"]), 1.0, None,
                                 A.mult)
                gp.tensor_scalar(s4[:, :, 1:, 11], r3(s["fxD"]), 1.0, None,
                                 A.mult)
                x0b = x0s[:, None, 0:12].broadcast_to([P, NG, 12])
                gp.tensor_scalar(s4[:, :, 0, :], x0b, 1.0, None, A.mult)

                # DVE speed scan + trig chain
                v.tensor_tensor_scan(s2pad[:, g * CFP:(g + 1) * CFP],
                                     apad[:, g * CFP:(g + 1) * CFP],
                                     zeros[:], 0.0, A.add, A.max)
                s2v = s2p3[:, g * NG:(g + 1) * NG, 1:]

                # cosd first (rc needs only it); ordered so chunk 0's sins
                # follow the last tanh immediately, and later chunks' sins
                # queue behind chunk 0's arctan (keeping the one table load
                # as early as possible in the ACT stream)
                cosd = sh.tile([P, CF], f32, tag="cosd")
                i_cd = sc.activation(cosd[:], s["dc"][:], AF.Sin, bias=hpiv)
                sind = sh.tile([P, CF], f16, tag="sind")
                i_sd = sc.activation(sind[:], s["dc"][:], AF.Sin)
                add_dep_helper(i_cd.ins, last_w1_act.ins, reason="act order")
                add_dep_helper(i_sd.ins, last_w1_act.ins, reason="act order")

                rc = sh.tile([P, CF], f32, tag="rc")
                v.reciprocal_approx_fast(rc[:], cosd[:])
                t45c = sh.tile([P, CF], f16, tag="t45c")
                v.scalar_tensor_tensor(t45c[:], sind[:], 0.45 / 12.6,
                                       rc[:], A.mult, A.mult)

                # yaw chain
                mch = sh.tile([P, CF], f32, tag="mch")
                gp.tensor_scalar(r3(mch), s2v, 0.1, 2.0, A.mult, A.max)
                imc = sh.tile([P, CF], f32, tag="imc")
                v.reciprocal_approx_fast(imc[:], mch[:])
                ylc = sh.tile([P, CF], f16, tag="ylc")
                gp.tensor_scalar(ylc[:], imc[:], f981, 0.15, A.mult, A.max)
                rawc = sh.tile([P, CF], f16, tag="rawc")
                v.tensor_tensor(r3(rawc), s2v, r3(t45c), A.mult)
                clpc = sh.tile([P, CF], f16, tag="clpc")
                gp.tensor_scalar(clpc[:], rawc[:], 1.0, -1.0, A.min, A.max)
                yawD = lg2.tile([P, CF], f16, tag="yawD")
                v.tensor_tensor(yawD[:], clpc[:], ylc[:], A.mult)
                yawP = sh.tile([P, CF], f16, tag="yawP")
                v.tensor_scalar(yawP[:], yawD[:], 1.0, None, A.mult)
                yp3 = r3(yawP)
                v.tensor_scalar(yp3[:, :, 0:1], yp3[:, :, 0:1], psi010,
                                None, A.add)

                # heading cumsum, psi, beta, q, wraps, sin/cos
                Pp = lg2.tile([P, CF], f16, tag="Pp")
                v.tensor_tensor_scan(Pp[:], maskc[:], yawP[:], 0.0,
                                     A.mult, A.add)
                sc.activation(s4[:, :, 1:, 2], r3(Pp), AF.Identity, scale=DT)
                btD = lg2.tile([P, CF], f16, tag="btD")
                i_bt = sc.activation(btD[:], t45c[:], AF.Arctan, scale=12.6)
                add_dep_helper(i_bt.ins, last_w1_act.ins,
                               reason="act table order tanh->trig")
                if first_bt[0] is None:
                    first_bt[0] = i_bt
                # chunk 0's post-beta chain gates the first output DMA:
                # boost it (only this tail - boosting the pre-beta part
                # starves wave 1 of later chunks and delays the barrier)
                import contextlib
                boost = tc.high_priority() if g == 0 else contextlib.nullcontext()
                with boost:
                    _emit_tail(g, s, s4, stg, s2v, Pp, btD, yawD)

            def _emit_tail(g, s, s4, stg, s2v, Pp, btD, yawD):
                q = sh.tile([P, CF], f32, tag="q")
                v.scalar_tensor_tensor(q[:], Pp[:], DT, btD[:],
                                       A.mult, A.add)
                # sin path first: the y-chain (vy -> Fy -> py / ay) is the
                # longest terminal tail
                wsc = sh.tile([P, 2 * CF], f32, tag="wsc")
                v.add_range_wrap(wsc[:, 0:CF], q[:], 0.0, PI, 2 * PI)
                snc = sh.tile([P, 2 * CF], f16, tag="snc")
                sc.activation(snc[:, 0:CF], wsc[:, 0:CF], AF.Sin, scale=SINSC)
                v.add_range_wrap(wsc[:, CF:2 * CF], q[:], PI / 2, PI, 2 * PI)
                sc.activation(snc[:, CF:2 * CF], wsc[:, CF:2 * CF], AF.Sin,
                              scale=SINSC)
                sinA = snc[:, 0:CF]
                cosA = snc[:, CF:2 * CF]

                # velocities (x10) + position cumsums (y first: the Pool ay
                # diff is the longest terminal, start it earliest)
                vyD = lg2.tile([P, CF], f16, tag="vyD")
                v.tensor_tensor(r3(vyD), s2v,
                                sinA.rearrange("p (n h) -> p n h", n=NG),
                                A.mult)
                vxD = lg2.tile([P, CF], f16, tag="vxD")
                v.tensor_tensor(r3(vxD), s2v,
                                cosA.rearrange("p (n h) -> p n h", n=NG),
                                A.mult)
                Fx = lg2.tile([P, CF], f16, tag="Fx")
                v.tensor_tensor_scan(Fx[:], maskc[:], vxD[:], 0.0,
                                     A.mult, A.add)
                Fy = lg2.tile([P, CF], f16, tag="Fy")
                v.tensor_tensor_scan(Fy[:], maskc[:], vyD[:], 0.0,
                                     A.mult, A.add)

                # terminals
                vxD3, vyD3 = r3(vxD), r3(vyD)
                sc.activation(s4[:, :, 1:, 8], r3(btD), AF.Copy)
                sc.activation(s4[:, :, 1:, 3], vxD3, AF.Copy, scale=0.1)
                sc.activation(s4[:, :, 1:, 0], r3(Fx), AF.Identity,
                              bias=px0, scale=DT * 0.1)
                sc.activation(s4[:, :, 1:, 1], r3(Fy), AF.Identity,
                              bias=py0, scale=DT * 0.1)
                v.tensor_tensor(s4[:, :, 2:, 6], vxD3[:, :, 1:],
                                vxD3[:, :, :H - 1], A.subtract)
                v.tensor_scalar(s4[:, :, 1, 6], vxD3[:, :, 0], 1.0, vx010,
                                A.mult, A.subtract)
                gp.tensor_scalar(s4[:, :, 1:, 5], r3(yawD), 1.0, None,
                                 A.mult)
                gp.tensor_scalar(s4[:, :, 1:, 4], vyD3, 0.1, None, A.mult)
                gp.tensor_tensor(s4[:, :, 2:, 7], vyD3[:, :, 1:],
                                 vyD3[:, :, :H - 1], A.subtract)
                gp.tensor_scalar(s4[:, :, 1, 7], vyD3[:, :, 0], 1.0, vy010,
                                 A.mult, A.subtract)

                sy.dma_start(out=out_d[:, g * CHW:(g + 1) * CHW], in_=stg[:])

            # emission order: chunks 0/1 (and their wave-1 DVE clusters)
            # first so the DVE stream serves the first two output DMAs
            # before chunks 2/3 consume any DVE time
            emit_w1d(0)
            emit_w1d(1)
            with tc.high_priority():
                emit_chunk(0)
            emit_chunk(1)
            emit_w1d(2)
            emit_chunk(2)
            emit_w1d(3)
            emit_chunk(3)

    bacc.get_activation_tables = _patched_tables
    try:
        nc.compile()
    finally:
        bacc.get_activation_tables = _orig_tables
    return nc


def _get_built():
    global _BUILT
    if _BUILT is None:
        _BUILT = _build_kernel()
    return _BUILT


def _run(x0, controls, deg, trace=False):
    from concourse.bass_utils import run_bass_kernel_spmd

    x0 = np.ascontiguousarray(x0, dtype=np.float32)
    controls = np.ascontiguousarray(controls, dtype=np.float32)
    deg = np.ascontiguousarray(deg, dtype=np.float32)

    nc = _get_built()
    in_maps = []
    for c in range(NCORES):
        sl = slice(c * BC, (c + 1) * BC)
        # [P, NPT, H, 3] -> per chunk channel-major [P, G, 3, NG, H];
        # u1/u2 prescaled by 0.5 (tanh half-angle sigmoid) so wave 1 is a
        # single full-scale tanh per chunk
        ctrl_c = controls[sl].reshape(R, H, 3).reshape(P, G, NG, H, 3)
        ctrl_c = ctrl_c.transpose(0, 1, 4, 2, 3).copy()   # [P, G, 3, NG, H]
        ctrl_c[:, :, 1:3] *= 0.5
        ctrl_c = ctrl_c.reshape(P, NPT * H * 3)
        x0p = np.repeat(x0[sl], P // BC, axis=0)      # [128, 12]
        degp = np.repeat(deg[sl], P // BC, axis=0)    # [128, 5]
        x0f = x0p.astype(np.float64)
        scal = np.zeros((P, 16), dtype=np.float32)
        fric = np.maximum(degp[:, 4], 0.1)
        scal[:, 0] = np.maximum(degp[:, 0], 0.05)            # steer
        scal[:, 3] = -7.5 * fric                             # lo (x10)
        scal[:, 4] = 9.81 * fric                             # f981
        scal[:, 5] = 10.0 * np.sqrt(x0f[:, 3] ** 2 + x0f[:, 4] ** 2 + 1e-6)
        scal[:, 7] = x0p[:, 0]                               # px0
        scal[:, 8] = x0p[:, 1]                               # py0
        scal[:, 9] = 10.0 * x0p[:, 3]                        # 10*vx0
        scal[:, 10] = 10.0 * x0p[:, 4]                       # 10*vy0
        scal[:, 11] = x0p[:, 2] / DT                         # psi0/DT
        scal[:, 12] = 0.5 * np.maximum(degp[:, 1], 0.05)     # brake/2
        scal[:, 13] = 0.5 * np.maximum(degp[:, 2], 0.05)     # thr/2
        scal[:, 14] = np.pi / 2
        in_maps.append({
            "ctrl": np.ascontiguousarray(ctrl_c.astype(np.float16)),
            "x0p": np.ascontiguousarray(x0p),
            "scal": scal,
        })

    res = run_bass_kernel_spmd(nc, in_maps, list(range(NCORES)), trace=trace)
    outs = []
    for c in range(NCORES):
        o = np.asarray(res.results[c]["out"])
        outs.append(o.reshape(R, HP1, CW).reshape(BC, L, HP1, CW))
    return np.concatenate(outs, axis=0), res


def kernel(x0: np.ndarray, controls: np.ndarray, deg: np.ndarray) -> np.ndarray:
    out, _ = _run(x0, controls, deg)
    return out


if __name__ == "__main__":
    rng = np.random.default_rng(0)
    x0 = rng.standard_normal((B, 12)).astype(np.float32)
    controls = rng.standard_normal((B, L, H, 3)).astype(np.float32)
    deg = rng.random((B, 5)).astype(np.float32)
    out = kernel(x0, controls, deg)
    print("out", out.shape, out.dtype)
